# revision 7
# baseline (speedup 1.0000x reference)
"""Trainium2 Bass kernel for nn_MultiModalFusionModelWithAblation.

Strategy: pure data-parallel over 8 NeuronCores (B=16384 -> 2048 rows/core).
Row-major home layout ([rows<=128 partitions, features free]); all matmuls
take the activation as the stationary operand (lhsT, feature-major via bf16
DMA-transpose) and the weight as the moving operand, so outputs land
row-major in PSUM.  bf16 matmul inputs, fp32 PSUM accumulation.

Host-side algebra (exact, weight-space only):
  - gat_W folded into the MHA score/value projections: Wh/ctx/k are never
    materialized.  scores use GS = gat_W @ [A_emo|A_pkl|a1|a2] (A from the
    per-head query vectors, incl. 1/sqrt(HD)); values use GV = gat_W @ Wv.
  - LN affines folded into downstream weights where linear.
  - all per-output-feature biases handled by K=1 ones-outer-product matmuls
    into PSUM (skipped entirely when the bias is zero, the spec default).
"""
import sys
import os

sys.path.insert(0, "/opt/trn_rl_repo")

import numpy as np
import orjson
from contextlib import ExitStack

import concourse.bass as bass
import concourse.tile as tile
from concourse import mybir
from concourse.masks import make_identity

# ----------------------------------------------------------------------------
# walrus on this toolchain rejects >1 sync-wait per instruction; split excess
# waits onto NoOp carriers on the same engine queue (in-order => equivalent).
_FIXN = [0]


def _fix_bir_waits(d):
    for f in d.get("functions", []):
        for b in f.get("blocks", []):
            insts = b.get("instructions", [])
            if not any(
                len(((i.get("sync_info") or {}).get("on_wait") or [])) > 1
                for i in insts
            ):
                continue
            new = []
            for inst in insts:
                si = inst.get("sync_info")
                waits = (si or {}).get("on_wait") or []
                if len(waits) > 1:
                    for w in waits[:-1]:
                        _FIXN[0] += 1
                        new.append({
                            "engine": inst["engine"], "ins": [], "outs": [],
                            "name": f"wfix-{_FIXN[0]}", "opcode": "NoOp",
                            "debug": inst.get("debug", 0),
                            "sync_info": {"on_update": [], "on_wait": [w]},
                        })
                    si["on_wait"] = [waits[-1]]
                new.append(inst)
            b["instructions"] = new
    return d


if not getattr(bass.Bass, "_waitfix_installed", False):
    _orig_tjb = bass.Bass.to_json_bytes

    def _patched_tjb(self):
        return orjson.dumps(_fix_bir_waits(orjson.loads(_orig_tjb(self))))

    bass.Bass.to_json_bytes = _patched_tjb
    bass.Bass._waitfix_installed = True

# ----------------------------------------------------------------------------
H = 512
NH = 8
HD = 64
NMOD = 5
IN_DIMS = [2048, 1024, 1536, 512, 512]
MODS = ["body", "face", "scene", "audio", "text"]
B_FULL = 16384
NCORES = 8
B_CORE = B_FULL // NCORES          # 2048
NT = B_CORE // 128                 # 16 row tiles per core
ALPHA = 0.2
EPS = 1e-5

F32 = mybir.dt.float32
BF16 = mybir.dt.bfloat16
AF = mybir.ActivationFunctionType
AL = mybir.AluOpType


def _build_nc(flags):
    """Build the SPMD per-core Bass program. `flags` has booleans for the
    optional bias paths (all False for the spec's zero-filled biases)."""
    nc = bass.Bass("TRN2", target_bir_lowering=False, debug=False,
                   num_devices=NCORES)

    # ---- dram io ----
    feat_d = [nc.dram_tensor(f"feat_{m}", [B_CORE, ind], F32, kind="ExternalInput")
              for m, ind in zip(MODS, IN_DIMS)]
    wp_d = [nc.dram_tensor(f"wp_{m}", [ind, H], F32, kind="ExternalInput")
            for m, ind in zip(MODS, IN_DIMS)]
    aw1_d = nc.dram_tensor("aw1", [NMOD, H, H // 2], F32, kind="ExternalInput")
    aw2_d = nc.dram_tensor("aw2", [NMOD, H // 2, H], F32, kind="ExternalInput")
    gv_d = nc.dram_tensor("gv", [H, H], F32, kind="ExternalInput")
    gs_d = nc.dram_tensor("gs", [H, 18], F32, kind="ExternalInput")
    wo_d = nc.dram_tensor("wo", [H, H], F32, kind="ExternalInput")
    pc_d = nc.dram_tensor("pc", [H, 24], F32, kind="ExternalInput")
    elp5_d = nc.dram_tensor("elp5", [35, H], F32, kind="ExternalInput")
    plp5_d = nc.dram_tensor("plp5", [25, H], F32, kind="ExternalInput")
    logits_d = nc.dram_tensor("logits", [NMOD, B_CORE, 7], F32, kind="ExternalInput")
    scores5_d = nc.dram_tensor("scores5", [NMOD, B_CORE, 5], F32, kind="ExternalInput")
    # optional bias rows (always declared; tiny)
    bp_d = nc.dram_tensor("bp", [NMOD, H], F32, kind="ExternalInput")
    ab1_d = nc.dram_tensor("ab1e", [NMOD, H // 2], F32, kind="ExternalInput")
    ab2_d = nc.dram_tensor("ab2e", [NMOD, H], F32, kind="ExternalInput")
    rc_d = nc.dram_tensor("rc", [2, H], F32, kind="ExternalInput")
    pcb_d = nc.dram_tensor("pcb", [1, 24], F32, kind="ExternalInput")
    ck_d = nc.dram_tensor("ck", [1, 16], F32, kind="ExternalInput")
    out_d = nc.dram_tensor("out", [B_CORE, 12], F32, kind="ExternalOutput")

    NK = [ind // 128 for ind in IN_DIMS]

    with tile.TileContext(nc) as tc, ExitStack() as ctx:
        wp_pool = ctx.enter_context(tc.tile_pool(name="weights", bufs=1))
        sb = ctx.enter_context(tc.tile_pool(name="work", bufs=1))
        ps = ctx.enter_context(tc.tile_pool(name="psum", bufs=1, space="PSUM"))

        # ---- one-time weight loads (gpsimd DMA casts fp32->bf16 in flight) --
        wp_bf = []
        for m in range(NMOD):
            t = wp_pool.tile([128, NK[m], H], BF16, tag=f"wp{m}")
            nc.gpsimd.dma_start(
                t[:], wp_d[m].ap().rearrange("(k c) n -> c k n", c=128))
            wp_bf.append(t)
        aw1_bf = wp_pool.tile([128, NMOD, 4, H // 2], BF16, tag="aw1")
        nc.gpsimd.dma_start(
            aw1_bf[:], aw1_d.ap().rearrange("m (k c) n -> c m k n", c=128))
        aw2_bf = wp_pool.tile([128, NMOD, 2, H], BF16, tag="aw2")
        nc.gpsimd.dma_start(
            aw2_bf[:], aw2_d.ap().rearrange("m (k c) n -> c m k n", c=128))
        gv_bf = wp_pool.tile([128, 4, H], BF16, tag="gv")
        nc.gpsimd.dma_start(gv_bf[:], gv_d.ap().rearrange("(k c) n -> c k n", c=128))
        gs_bf = wp_pool.tile([128, 4, 18], BF16, tag="gs")
        nc.gpsimd.dma_start(gs_bf[:], gs_d.ap().rearrange("(k c) n -> c k n", c=128))
        wo_bf = wp_pool.tile([128, 4, H], BF16, tag="wo")
        nc.gpsimd.dma_start(wo_bf[:], wo_d.ap().rearrange("(k c) n -> c k n", c=128))
        pc_bf = wp_pool.tile([128, 4, 24], BF16, tag="pc")
        nc.gpsimd.dma_start(pc_bf[:], pc_d.ap().rearrange("(k c) n -> c k n", c=128))
        elp5_bf = wp_pool.tile([35, H], BF16, tag="elp5")
        nc.gpsimd.dma_start(elp5_bf[:], elp5_d.ap()[:])
        plp5_bf = wp_pool.tile([25, H], BF16, tag="plp5")
        nc.gpsimd.dma_start(plp5_bf[:], plp5_d.ap()[:])

        ident = wp_pool.tile([128, 128], BF16, tag="ident")
        make_identity(nc, ident[:])
        eps_t = wp_pool.tile([128, 1], F32, tag="eps")
        nc.vector.memset(eps_t[:], EPS)

        ones1 = None
        if any([flags["bp"], flags["ab1"], flags["ab2"], flags["rc"],
                flags["pcb"], flags["ck"]]):
            ones1 = wp_pool.tile([1, 128], BF16, tag="ones1")
            nc.vector.memset(ones1[:], 1.0)

        def _bias_row(dram_ap, n, tag):
            t = wp_pool.tile([1, n], BF16, tag=tag)
            nc.gpsimd.dma_start(t[:], dram_ap)
            return t

        bp_bf = _bias_row(bp_d.ap().rearrange("m n -> 1 (m n)"), NMOD * H, "bp") \
            if flags["bp"] else None
        ab1_bf = _bias_row(ab1_d.ap().rearrange("m n -> 1 (m n)"), NMOD * 256, "ab1") \
            if flags["ab1"] else None
        ab2_bf = _bias_row(ab2_d.ap().rearrange("m n -> 1 (m n)"), NMOD * H, "ab2") \
            if flags["ab2"] else None
        rc_bf = _bias_row(rc_d.ap().rearrange("q n -> 1 (q n)"), 2 * H, "rc") \
            if flags["rc"] else None
        pcb_bf = _bias_row(pcb_d.ap()[:], 24, "pcb") if flags["pcb"] else None
        ck_t = None
        if flags["ck"]:
            ck_row = _bias_row(ck_d.ap()[:], 16, "ckrow")
            ck_ps = ps.tile([128, 16], F32, tag="psB")
            nc.tensor.matmul(ck_ps[:], lhsT=ones1[:], rhs=ck_row[:],
                             start=True, stop=True)
            ck_t = wp_pool.tile([128, 16], F32, tag="ckt")
            nc.vector.tensor_copy(out=ck_t[:], in_=ck_ps[:])

        # ---------------- per row-tile pipeline ----------------
        for rt in range(NT):
            r0 = rt * 128
            xss = sb.tile([128, NMOD, 18], F32, tag="xss", bufs=2)
            xvt = sb.tile([128, H, NMOD], BF16, tag="xvt", bufs=2)

            for m in range(NMOD):
                nk = NK[m]
                # load + cast feat tile
                fz = sb.tile([128, IN_DIMS[0]], BF16, tag="fz", bufs=3)
                nc.gpsimd.dma_start(
                    fz[:, :IN_DIMS[m]], feat_d[m].ap()[r0:r0 + 128, :])
                # transpose to feature-major
                fT = sb.tile([128, NK[0], 128], BF16, tag="fT", bufs=2)
                for k in range(nk):
                    nc.sync.dma_start(fT[:, k, :], fz[:, k * 128:(k + 1) * 128],
                                      transpose=True)
                # projection
                h_ps = ps.tile([128, H], F32, tag="psA", bufs=2)
                if flags["bp"]:
                    nc.tensor.matmul(h_ps[:], lhsT=ones1[:],
                                     rhs=bp_bf[:, m * H:(m + 1) * H],
                                     start=True, stop=False)
                for k in range(nk):
                    nc.tensor.matmul(h_ps[:], lhsT=fT[:, k, :], rhs=wp_bf[m][:, k, :],
                                     start=(k == 0 and not flags["bp"]),
                                     stop=(k == nk - 1))
                # relu + LN1
                h_sb = sb.tile([128, H], BF16, tag="h_sb", bufs=2)
                nc.scalar.activation(h_sb[:], h_ps[:], AF.Relu)
                st6 = sb.tile([128, 6], F32, tag="st6", bufs=2)
                nc.vector.bn_stats(st6[:], h_sb[:])
                st2 = sb.tile([128, 2], F32, tag="st2", bufs=2)
                nc.vector.bn_aggr(st2[:], st6[:])
                sd = sb.tile([128, 1], F32, tag="sd", bufs=2)
                nc.scalar.activation(sd[:], st2[:, 1:2], AF.Sqrt, bias=eps_t[:])
                rs = sb.tile([128, 1], F32, tag="rs", bufs=2)
                nc.vector.reciprocal(rs[:], sd[:])
                hln = sb.tile([128, H], BF16, tag="hln", bufs=2)
                nc.vector.tensor_scalar(out=hln[:], in0=h_sb[:],
                                        scalar1=st2[:, 0:1], scalar2=rs[:],
                                        op0=AL.subtract, op1=AL.mult)
                # adapter
                hT = sb.tile([128, 4, 128], BF16, tag="hT", bufs=2)
                for k in range(4):
                    nc.sync.dma_start(hT[:, k, :], hln[:, k * 128:(k + 1) * 128],
                                      transpose=True)
                a1_ps = ps.tile([128, 256], F32, tag="psB", bufs=2)
                if flags["ab1"]:
                    nc.tensor.matmul(a1_ps[:], lhsT=ones1[:],
                                     rhs=ab1_bf[:, m * 256:(m + 1) * 256],
                                     start=True, stop=False)
                for k in range(4):
                    nc.tensor.matmul(a1_ps[:], lhsT=hT[:, k, :],
                                     rhs=aw1_bf[:, m, k, :],
                                     start=(k == 0 and not flags["ab1"]),
                                     stop=(k == 3))
                z = sb.tile([128, 256], BF16, tag="z", bufs=2)
                nc.scalar.activation(z[:], a1_ps[:], AF.Relu)
                zT = sb.tile([128, 2, 128], BF16, tag="zT", bufs=2)
                for k in range(2):
                    nc.sync.dma_start(zT[:, k, :], z[:, k * 128:(k + 1) * 128],
                                      transpose=True)
                a2_ps = ps.tile([128, H], F32, tag="psA", bufs=2)
                if flags["ab2"]:
                    nc.tensor.matmul(a2_ps[:], lhsT=ones1[:],
                                     rhs=ab2_bf[:, m * H:(m + 1) * H],
                                     start=True, stop=False)
                for k in range(2):
                    nc.tensor.matmul(a2_ps[:], lhsT=zT[:, k, :],
                                     rhs=aw2_bf[:, m, k, :],
                                     start=(k == 0 and not flags["ab2"]),
                                     stop=(k == 1))
                # residual + LN2
                u = sb.tile([128, H], BF16, tag="u", bufs=2)
                nc.vector.tensor_tensor(out=u[:], in0=a2_ps[:], in1=hln[:],
                                        op=AL.add)
                st6b = sb.tile([128, 6], F32, tag="st6b", bufs=2)
                nc.vector.bn_stats(st6b[:], u[:])
                st2b = sb.tile([128, 2], F32, tag="st2b", bufs=2)
                nc.vector.bn_aggr(st2b[:], st6b[:])
                sdb = sb.tile([128, 1], F32, tag="sdb", bufs=2)
                nc.scalar.activation(sdb[:], st2b[:, 1:2], AF.Sqrt, bias=eps_t[:])
                rsb = sb.tile([128, 1], F32, tag="rsb", bufs=2)
                nc.vector.reciprocal(rsb[:], sdb[:])
                xm = sb.tile([128, H], BF16, tag="xm", bufs=2)
                nc.vector.tensor_scalar(out=xm[:], in0=u[:],
                                        scalar1=st2b[:, 0:1], scalar2=rsb[:],
                                        op0=AL.subtract, op1=AL.mult)
                # graph/value/score projections
                xT = sb.tile([128, 4, 128], BF16, tag="xT", bufs=2)
                for k in range(4):
                    nc.sync.dma_start(xT[:, k, :], xm[:, k * 128:(k + 1) * 128],
                                      transpose=True)
                xv_ps = ps.tile([128, H], F32, tag="psC", bufs=2)
                for k in range(4):
                    nc.tensor.matmul(xv_ps[:], lhsT=xT[:, k, :], rhs=gv_bf[:, k, :],
                                     start=(k == 0), stop=(k == 3))
                xs_ps = ps.tile([128, 18], F32, tag="psB", bufs=2)
                for k in range(4):
                    nc.tensor.matmul(xs_ps[:], lhsT=xT[:, k, :], rhs=gs_bf[:, k, :],
                                     start=(k == 0), stop=(k == 3))
                nc.scalar.activation(xvt[:, :, m], xv_ps[:], AF.Copy)
                nc.vector.tensor_copy(out=xss[:, m, :], in_=xs_ps[:])

            # ---- GAT attention over modalities (all-ones adjacency) ----
            e = sb.tile([128, 5, 5], F32, tag="e", bufs=2)
            s2cat = xss[:, :, 17]
            for i in range(NMOD):
                nc.vector.tensor_scalar(out=e[:, i, :], in0=s2cat,
                                        scalar1=xss[:, i, 16:17], scalar2=None,
                                        op0=AL.add)
            el = sb.tile([128, 25], F32, tag="el", bufs=2)
            nc.vector.tensor_scalar_mul(el[:], e[:].rearrange("p a b -> p (a b)"),
                                        ALPHA)
            nc.vector.tensor_tensor(out=el[:], in0=el[:],
                                    in1=e[:].rearrange("p a b -> p (a b)"),
                                    op=AL.max)
            ex = sb.tile([128, 5, 5], F32, tag="ex", bufs=2)
            nc.scalar.activation(ex[:].rearrange("p a b -> p (a b)"), el[:], AF.Exp)
            den = sb.tile([128, 5], F32, tag="den", bufs=2)
            nc.vector.tensor_reduce(out=den[:], in_=ex[:], axis=mybir.AxisListType.X,
                                    op=AL.add)
            rden = sb.tile([128, 5], F32, tag="rden", bufs=2)
            nc.vector.reciprocal(rden[:], den[:])
            attn = sb.tile([128, 5, 5], F32, tag="attn", bufs=2)
            nc.vector.tensor_tensor(
                out=attn[:], in0=ex[:],
                in1=rden[:, :, None].broadcast_to([128, 5, 5]), op=AL.mult)

            # ---- pooled attention scores s[b,(q,h),n] ----
            tmp400 = sb.tile([128, 16, 5, 5], F32, tag="tmp400", bufs=2)
            nc.vector.tensor_tensor(
                out=tmp400[:],
                in0=xss[:, :, 0:16].rearrange("p j q -> p q j")[:, :, None, :]
                    .broadcast_to([128, 16, 5, 5]),
                in1=attn[:][:, None, :, :].broadcast_to([128, 16, 5, 5]),
                op=AL.mult)
            S = sb.tile([128, 16, 5], F32, tag="S", bufs=2)
            nc.vector.tensor_reduce(out=S[:], in_=tmp400[:],
                                    axis=mybir.AxisListType.X, op=AL.add)
            if flags["ck"]:
                nc.vector.tensor_tensor(
                    out=S[:], in0=S[:],
                    in1=ck_t[:][:, :, None].broadcast_to([128, 16, 5]), op=AL.add)
            ES = sb.tile([128, 16, 5], F32, tag="ES", bufs=2)
            nc.scalar.activation(ES[:].rearrange("p a b -> p (a b)"),
                                 S[:].rearrange("p a b -> p (a b)"), AF.Exp)
            den16 = sb.tile([128, 16], F32, tag="den16", bufs=2)
            nc.vector.tensor_reduce(out=den16[:], in_=ES[:],
                                    axis=mybir.AxisListType.X, op=AL.add)
            rden16 = sb.tile([128, 16], F32, tag="rden16", bufs=2)
            nc.vector.reciprocal(rden16[:], den16[:])
            P = sb.tile([128, 16, 5], BF16, tag="P", bufs=2)
            nc.vector.tensor_tensor(
                out=P[:], in0=ES[:],
                in1=rden16[:, :, None].broadcast_to([128, 16, 5]), op=AL.mult)

            # ---- W[b,(q,h),j] = sum_n P * attn ----
            tmp2 = sb.tile([128, 16, 5, 5], BF16, tag="tmp2", bufs=2)
            nc.vector.tensor_tensor(
                out=tmp2[:],
                in0=P[:][:, :, None, :].broadcast_to([128, 16, 5, 5]),
                in1=attn[:].rearrange("p n j -> p j n")[:, None, :, :]
                    .broadcast_to([128, 16, 5, 5]),
                op=AL.mult)
            W = sb.tile([128, 16, 5], BF16, tag="W", bufs=2)
            with nc.allow_low_precision("5-term pooled-attn sums"):
                nc.vector.tensor_reduce(out=W[:], in_=tmp2[:],
                                        axis=mybir.AxisListType.X, op=AL.add)

            # ---- pooled values o_q = sum_j W ⊙ xv_j, out-proj, heads ----
            rep_bf, rnorm = [], []
            for q in range(2):
                tmp_o = sb.tile([128, NH, HD, 5], BF16, tag="tmp_o", bufs=2)
                nc.vector.tensor_tensor(
                    out=tmp_o[:],
                    in0=xvt[:].rearrange("p (h d) j -> p h d j", h=NH),
                    in1=W[:, q * 8:(q + 1) * 8, None, :]
                        .broadcast_to([128, NH, HD, 5]),
                    op=AL.mult)
                o_q = sb.tile([128, H], BF16, tag="o_q", bufs=2)
                with nc.allow_low_precision("5-term pooled-attn sums"):
                    nc.vector.tensor_reduce(
                        out=o_q[:].rearrange("p (h d) -> p h d", h=NH),
                        in_=tmp_o[:], axis=mybir.AxisListType.X, op=AL.add)
                oT = sb.tile([128, 4, 128], BF16, tag="oT", bufs=2)
                for k in range(4):
                    nc.sync.dma_start(oT[:, k, :], o_q[:, k * 128:(k + 1) * 128],
                                      transpose=True)
                repr_ps = ps.tile([128, H], F32, tag="psD", bufs=2)
                if flags["rc"]:
                    nc.tensor.matmul(repr_ps[:], lhsT=ones1[:],
                                     rhs=rc_bf[:, q * H:(q + 1) * H],
                                     start=True, stop=False)
                for k in range(4):
                    nc.tensor.matmul(repr_ps[:], lhsT=oT[:, k, :], rhs=wo_bf[:, k, :],
                                     start=(k == 0 and not flags["rc"]), stop=False)
                # aux-logit contribution via PE transpose + K<=35 matmul
                if q == 0:
                    lg = sb.tile([128, NMOD, 7], F32, tag="lg", bufs=2)
                    nc.gpsimd.dma_start(
                        lg[:], logits_d.ap()[:, r0:r0 + 128, :]
                        .rearrange("m r c -> r m c"))
                    lgb = sb.tile([128, 35], BF16, tag="lgb", bufs=2)
                    nc.vector.tensor_copy(
                        out=lgb[:], in_=lg[:].rearrange("p m c -> p (m c)"))
                    lt_ps = ps.tile([35, 128], BF16, tag="psB", bufs=2)
                    nc.tensor.transpose(lt_ps[:], lgb[:], ident[:])
                    lt = sb.tile([35, 128], BF16, tag="lt", bufs=2)
                    nc.vector.tensor_copy(out=lt[:], in_=lt_ps[:])
                    nc.tensor.matmul(repr_ps[:], lhsT=lt[:], rhs=elp5_bf[:],
                                     start=False, stop=True)
                else:
                    sc = sb.tile([128, NMOD, 5], F32, tag="sc", bufs=2)
                    nc.gpsimd.dma_start(
                        sc[:], scores5_d.ap()[:, r0:r0 + 128, :]
                        .rearrange("m r c -> r m c"))
                    scb = sb.tile([128, 25], BF16, tag="scb", bufs=2)
                    nc.vector.tensor_copy(
                        out=scb[:], in_=sc[:].rearrange("p m c -> p (m c)"))
                    st_ps = ps.tile([25, 128], BF16, tag="psB", bufs=2)
                    nc.tensor.transpose(st_ps[:], scb[:], ident[:])
                    st = sb.tile([25, 128], BF16, tag="st", bufs=2)
                    nc.vector.tensor_copy(out=st[:], in_=st_ps[:])
                    nc.tensor.matmul(repr_ps[:], lhsT=st[:], rhs=plp5_bf[:],
                                     start=False, stop=True)
                rb = sb.tile([128, H], BF16, tag=f"rep{q}", bufs=2)
                nc.scalar.activation(rb[:], repr_ps[:], AF.Copy)
                sq = sb.tile([128, H], BF16, tag="sq", bufs=2)
                n2 = sb.tile([128, 1], F32, tag=f"n2{q}", bufs=2)
                nc.vector.scalar_tensor_tensor(
                    out=sq[:], in0=rb[:], scalar=1.0, in1=repr_ps[:],
                    op0=AL.mult, op1=AL.mult, accum_out=n2[:])
                nrm = sb.tile([128, 1], F32, tag=f"nrm{q}", bufs=2)
                nc.scalar.activation(nrm[:], n2[:], AF.Sqrt)
                nc.vector.tensor_scalar_max(nrm[:], nrm[:], 1e-8)
                rn = sb.tile([128, 1], F32, tag=f"rn{q}", bufs=2)
                nc.vector.reciprocal(rn[:], nrm[:])
                rep_bf.append(rb)
                rnorm.append(rn)

            # ---- heads + cosine guides ----
            pred_ps = ps.tile([128, 24], F32, tag="psB", bufs=2)
            if flags["pcb"]:
                nc.tensor.matmul(pred_ps[:], lhsT=ones1[:], rhs=pcb_bf[:],
                                 start=True, stop=False)
            for q in range(2):
                rT = sb.tile([128, 4, 128], BF16, tag=f"rT{q}", bufs=2)
                for k in range(4):
                    nc.sync.dma_start(rT[:, k, :],
                                      rep_bf[q][:, k * 128:(k + 1) * 128],
                                      transpose=True)
                cols = slice(0, 14) if q == 0 else slice(14, 24)
                for k in range(4):
                    nc.tensor.matmul(pred_ps[:, cols], lhsT=rT[:, k, :],
                                     rhs=pc_bf[:, k, cols],
                                     start=(k == 0 and not flags["pcb"]),
                                     stop=(k == 3))
            pred = sb.tile([128, 24], F32, tag="pred", bufs=2)
            nc.vector.tensor_copy(out=pred[:], in_=pred_ps[:])

            outt = sb.tile([128, 12], F32, tag="outt", bufs=2)
            # emo_final = 0.5*pred + 0.5*cos*rnorm  (0.5 folded on host)
            nc.vector.scalar_tensor_tensor(
                out=outt[:, 0:7], in0=pred[:, 7:14], scalar=rnorm[0][:],
                in1=pred[:, 0:7], op0=AL.mult, op1=AL.add)
            sigc = sb.tile([128, 5], F32, tag="sigc", bufs=2)
            nc.scalar.activation(sigc[:], pred[:, 19:24], AF.Sigmoid,
                                 scale=rnorm[1][:])
            sigp = sb.tile([128, 5], F32, tag="sigp", bufs=2)
            nc.scalar.activation(sigp[:], pred[:, 14:19], AF.Sigmoid)
            sum5 = sb.tile([128, 5], F32, tag="sum5", bufs=2)
            nc.vector.tensor_tensor(out=sum5[:], in0=sigc[:], in1=sigp[:],
                                    op=AL.add)
            nc.vector.tensor_scalar_mul(outt[:, 7:12], sum5[:], 0.5)
            nc.sync.dma_start(out_d.ap()[r0:r0 + 128, :], outt[:])

    return nc


_CACHE = {}


def _host_prep(inputs):
    f32 = np.float32
    gat_W = inputs["gat_W"].astype(f32)
    gat_a = inputs["gat_a"].astype(f32)
    mha_in_w = inputs["mha_in_w"].astype(f32)
    mha_in_b = inputs["mha_in_b"].astype(f32)
    Wq, Wk, Wv = np.split(mha_in_w, 3, axis=1)
    bq, bk, bv = np.split(mha_in_b, 3)

    def score_mat(query):
        qv = (query.astype(f32) @ Wq + bq).reshape(NH, HD)
        A = np.stack([Wk[:, h * HD:(h + 1) * HD] @ qv[h] for h in range(NH)], 1)
        cK = np.array([bk[h * HD:(h + 1) * HD] @ qv[h] for h in range(NH)], f32)
        return A / np.sqrt(HD), cK / np.sqrt(HD)

    A_emo, ck_emo = score_mat(inputs["emo_query"])
    A_pkl, ck_pkl = score_mat(inputs["pkl_query"])
    gs = gat_W @ np.concatenate(
        [A_emo, A_pkl, gat_a[:H, None], gat_a[H:, None]], 1)
    gv = gat_W @ Wv
    ck = np.concatenate([ck_emo, ck_pkl]).astype(f32)

    ln1_g = inputs["ln1_g"].astype(f32)
    ln1_b = inputs["ln1_b"].astype(f32)
    ln2_g = inputs["ln2_g"].astype(f32)
    ln2_b = inputs["ln2_b"].astype(f32)
    ln1_trivial = np.allclose(ln1_g, 1.0) and np.allclose(ln1_b, 0.0)
    ln2_trivial = np.allclose(ln2_g, 1.0) and np.allclose(ln2_b, 0.0)
    if not (ln1_trivial and ln2_trivial):
        raise NotImplementedError("non-trivial LN affine not supported")

    aw1 = np.stack([np.diag(ln1_g[m]) @ inputs["aW1"][m].astype(f32)
                    for m in range(NMOD)])
    ab1e = inputs["ab1"].astype(f32) + np.einsum(
        "mk,mkn->mn", ln1_b, inputs["aW1"].astype(f32))
    aw2 = inputs["aW2"].astype(f32)
    ab2e = inputs["ab2"].astype(f32)

    mha_out_w = inputs["mha_out_w"].astype(f32)
    mha_out_b = inputs["mha_out_b"].astype(f32)
    rc = np.stack([
        mha_out_b + bv @ mha_out_w + inputs["elp_b"].astype(f32),
        mha_out_b + bv @ mha_out_w + inputs["plp_b"].astype(f32)])

    def norm_rows(g):
        g = g.astype(f32)
        n = np.maximum(np.linalg.norm(g, axis=-1, keepdims=True), 1e-8)
        return g / n

    gn_emo = norm_rows(inputs["guide_emo"])
    gn_pkl = norm_rows(inputs["guide_pkl"])
    pc = np.concatenate([
        inputs["emo_head_w"].astype(f32) * 0.5, gn_emo.T * 0.5,
        inputs["pkl_head_w"].astype(f32), gn_pkl.T], 1)
    pcb = np.concatenate([
        inputs["emo_head_b"].astype(f32) * 0.5, np.zeros(7, f32),
        inputs["pkl_head_b"].astype(f32), np.zeros(5, f32)])

    elp5 = np.tile(inputs["elp_w"].astype(f32) / NMOD, (NMOD, 1))
    plp5 = np.tile(inputs["plp_w"].astype(f32) / NMOD, (NMOD, 1))

    host = dict(
        gv=np.ascontiguousarray(gv, f32), gs=np.ascontiguousarray(gs, f32),
        wo=np.ascontiguousarray(mha_out_w, f32),
        pc=np.ascontiguousarray(pc, f32),
        elp5=np.ascontiguousarray(elp5, f32),
        plp5=np.ascontiguousarray(plp5, f32),
        aw1=np.ascontiguousarray(aw1, f32), aw2=np.ascontiguousarray(aw2, f32),
        bp=np.ascontiguousarray(inputs["bp"], f32),
        ab1e=np.ascontiguousarray(ab1e, f32),
        ab2e=np.ascontiguousarray(ab2e, f32),
        rc=np.ascontiguousarray(rc, f32),
        pcb=np.ascontiguousarray(pcb[None, :], f32),
        ck=np.ascontiguousarray(ck[None, :], f32),
    )
    flags = dict(
        bp=not np.allclose(host["bp"], 0.0),
        ab1=not np.allclose(host["ab1e"], 0.0),
        ab2=not np.allclose(host["ab2e"], 0.0),
        rc=not np.allclose(host["rc"], 0.0),
        pcb=not np.allclose(host["pcb"], 0.0),
        ck=not np.allclose(host["ck"], 0.0),
    )
    return host, flags


def _run(inputs, **spmd_kwargs):
    from concourse.bass_utils import run_bass_kernel_spmd

    host, flags = _host_prep(inputs)
    key = tuple(sorted(flags.items()))
    if key not in _CACHE:
        _CACHE[key] = _build_nc(flags)
    nc = _CACHE[key]

    in_maps = []
    for c in range(NCORES):
        r = slice(c * B_CORE, (c + 1) * B_CORE)
        im = {f"feat_{m}": np.ascontiguousarray(
                  inputs[f"feat_{m}"][r], np.float32) for m in MODS}
        for m in MODS:
            im[f"wp_{m}"] = np.ascontiguousarray(inputs[f"Wp_{m}"], np.float32)
        im["logits"] = np.ascontiguousarray(
            inputs["emo_logits_all"][:, r, :], np.float32)
        im["scores5"] = np.ascontiguousarray(
            inputs["per_scores_all"][:, r, :], np.float32)
        im.update(host)
        in_maps.append(im)

    res = run_bass_kernel_spmd(nc, in_maps, list(range(NCORES)), **spmd_kwargs)
    out = np.concatenate([res.results[c]["out"] for c in range(NCORES)], 0)
    return out, res


def kernel(**inputs):
    return _run(inputs)[0]
